# revision 3
# baseline (speedup 1.0000x reference)
"""HINN (hierarchical masked MLP) on 8 TRN2 NeuronCores.

Strategy (tensor-parallel, transposed layout):
  - All activations live as [features, batch] (features on partitions).
  - Y1/Y2: shard CPG (1000/core). Stationary = masked W_sc tile (natural
    [SNP, CPG] layout), moving = x_snp^T -> PSUM holds Y2^T directly.
    The tiny snp_fc20 head (20 cols) is folded into the stationary slab.
  - AllGather concatenates on the partition axis == feature axis: exactly
    reassembles Y2^T [8000, 512] from per-core shards.
  - fY2/Y3: shard GENE (250/core), K-loop over gathered Y2^T. cpg_fc20
    folded in as 20 extra stationary cols.
  - The 1/(fY2+eps) makes the Y1->fY2 chain numerically chaotic: it must
    run in true fp32 on the PE (4 cyc/row). Everything after the division
    (Y4, gene_fc, dense tail) runs in bf16.
  - Eval-mode BN is folded into the tail weights on the host.
  - The tail is computed replicated on every core; core 0's output is used.
"""
import sys
import numpy as np
import ml_dtypes

sys.path.insert(0, "/opt/trn_rl_repo")

import concourse.bacc as bacc
import concourse.bass as bass
import concourse.mybir as mybir
import concourse.tile as tile
from concourse.bass_utils import run_bass_kernel_spmd

BF16 = ml_dtypes.bfloat16
F32 = mybir.dt.float32
BF = mybir.dt.bfloat16

B = 512
SNP, CPG, GENE, GO = 10000, 8000, 2000, 1000
FC, DG = 20, 6
NCORES = 8
SH_C, SH_G, SH_O = CPG // NCORES, GENE // NCORES, GO // NCORES  # 1000, 250, 125
AW = SH_C + FC   # 1020 stationary cols in stage A
BW = SH_G + FC   # 270 stationary cols in stage B
CW = SH_O + FC   # 145 stationary cols in stage C
EPS = 1e-8
RG = [list(range(NCORES))]


def _ktiles(total, step=128):
    out = []
    off = 0
    while off < total:
        w = min(step, total - off)
        out.append((off, w))
        off += w
    return out


def _mtiles(total, step=128):
    return _ktiles(total, step)


def build_nc(trace=False):
    nc = bacc.Bacc("TRN2", target_bir_lowering=False, debug=False)

    # ---- DRAM parameters (per-core shards fed via in_maps) ----
    d_xT = nc.dram_tensor("xT", [SNP, B], F32, kind="ExternalInput")
    d_aw = nc.dram_tensor("aw", [SNP, AW], F32, kind="ExternalInput")
    d_am = nc.dram_tensor("am", [SNP, AW], F32, kind="ExternalInput")
    d_ab = nc.dram_tensor("ab", [AW, 1], F32, kind="ExternalInput")
    d_xcpgT = nc.dram_tensor("xcpgT", [SH_C, B], F32, kind="ExternalInput")
    d_gcpg = nc.dram_tensor("gcpg", [SH_C, 1], F32, kind="ExternalInput")
    d_vb2 = nc.dram_tensor("vb2", [SH_C, 1], F32, kind="ExternalInput")
    d_bw = nc.dram_tensor("bw", [CPG, BW], F32, kind="ExternalInput")
    d_bm = nc.dram_tensor("bm", [CPG, BW], F32, kind="ExternalInput")
    d_bb = nc.dram_tensor("bb", [BW, 1], F32, kind="ExternalInput")
    d_bw2 = nc.dram_tensor("bw2", [FC, FC], F32, kind="ExternalInput")
    d_xgeneT = nc.dram_tensor("xgeneT", [SH_G, B], F32, kind="ExternalInput")
    d_gg = nc.dram_tensor("gg", [SH_G, 1], F32, kind="ExternalInput")
    d_vb3 = nc.dram_tensor("vb3", [SH_G, 1], F32, kind="ExternalInput")
    d_cw = nc.dram_tensor("cw", [GENE, CW], BF, kind="ExternalInput")
    d_cm = nc.dram_tensor("cm", [GENE, CW], BF, kind="ExternalInput")
    d_cb = nc.dram_tensor("cb", [CW, 1], F32, kind="ExternalInput")
    d_cw2 = nc.dram_tensor("cw2", [FC, FC], BF, kind="ExternalInput")
    d_w1 = nc.dram_tensor("w1", [GO + FC, 128], BF, kind="ExternalInput")
    d_tb1 = nc.dram_tensor("tb1", [128, 1], F32, kind="ExternalInput")
    d_w2 = nc.dram_tensor("w2", [128, 128], BF, kind="ExternalInput")
    d_tb2 = nc.dram_tensor("tb2", [128, 1], F32, kind="ExternalInput")
    d_w3 = nc.dram_tensor("w3", [128, 128], BF, kind="ExternalInput")
    d_tb3 = nc.dram_tensor("tb3", [128, 1], F32, kind="ExternalInput")
    d_w4 = nc.dram_tensor("w4", [128, 128], BF, kind="ExternalInput")
    d_tb4 = nc.dram_tensor("tb4", [128, 1], F32, kind="ExternalInput")
    d_wp = nc.dram_tensor("wp", [128, FC], BF, kind="ExternalInput")
    d_tbp = nc.dram_tensor("tbp", [FC, 1], F32, kind="ExternalInput")
    d_demogT = nc.dram_tensor("demogT", [DG, B], BF, kind="ExternalInput")
    d_wd = nc.dram_tensor("wd", [FC + DG, 128], BF, kind="ExternalInput")
    d_tbd = nc.dram_tensor("tbd", [128, 1], F32, kind="ExternalInput")
    d_wo = nc.dram_tensor("wo", [128, 1], BF, kind="ExternalInput")
    d_tbo = nc.dram_tensor("tbo", [1, 1], F32, kind="ExternalInput")
    d_out = nc.dram_tensor("out", [1, B], F32, kind="ExternalOutput")

    KA = _ktiles(SNP)    # 79 tiles
    KB = _ktiles(CPG)    # 63 tiles
    KC = _ktiles(GENE)   # 16 tiles
    MA = _mtiles(AW)     # 8 tiles: 7x128 + 124
    MB = [(0, 128), (128, SH_G - 128), (SH_G, FC)]   # 128, 122, 20
    MC = [(0, SH_O), (SH_O, FC)]                      # 125, 20

    mul = mybir.AluOpType.mult
    Relu = mybir.ActivationFunctionType.Relu
    Ident = mybir.ActivationFunctionType.Identity

    with tile.TileContext(nc) as tc:
        with (
            tc.tile_pool(name="wpool", bufs=3) as wpool,
            tc.tile_pool(name="mpool", bufs=3) as mpool,
            tc.tile_pool(name="rhs", bufs=4) as rhsp,
            tc.tile_pool(name="ev", bufs=4) as evp,
            tc.tile_pool(name="small", bufs=4) as smallp,
            tc.tile_pool(name="persist", bufs=1) as pers,
            tc.tile_pool(name="psum", bufs=1, space="PSUM") as pp,
            tc.tile_pool(name="dram", bufs=1, space="DRAM") as dp,
        ):
            # ---------------- Stage A: Y1/Y2 (+snp_fc), fp32 ----------------
            psA = [pp.tile([128, B], F32, tag=f"pb{j}", name=f"psA{j}")
                   for j in range(8)]
            for ki, (ko, kw) in enumerate(KA):
                wt = wpool.tile([128, AW], F32, name="wt")
                mt = mpool.tile([128, AW], F32, name="mt")
                nc.sync.dma_start(wt[:kw], d_aw[ko:ko + kw, :])
                nc.sync.dma_start(mt[:kw], d_am[ko:ko + kw, :])
                nc.vector.tensor_mul(wt[:kw], wt[:kw], mt[:kw])
                xt = rhsp.tile([128, B], F32, name="xt")
                nc.sync.dma_start(xt[:kw], d_xT[ko:ko + kw, :])
                st = ki == 0
                sp = ki == len(KA) - 1
                for j, (mo, mw) in enumerate(MA):
                    nc.tensor.matmul(psA[j][:mw], wt[:kw, mo:mo + mw], xt[:kw],
                                     start=st, stop=sp)

            y2sh = dp.tile([SH_C, B], F32, name="y2sh")
            snp_fc = pers.tile([FC, B], F32, name="snp_fc")
            for j, (mo, mw) in enumerate(MA):
                ba = smallp.tile([128, 1], F32, name="ba")
                nc.sync.dma_start(ba[:mw], d_ab[mo:mo + mw, :])
                y1 = evp.tile([128, B], F32, name="y1")
                nc.scalar.activation(y1[:mw], psA[j][:mw], Relu, bias=ba[:mw])
                ncpg = min(mw, SH_C - mo) if mo < SH_C else 0
                if ncpg > 0:
                    xc = evp.tile([128, B], F32, name="xc")
                    nc.sync.dma_start(xc[:ncpg], d_xcpgT[mo:mo + ncpg, :])
                    gt = smallp.tile([128, 1], F32, name="gt")
                    nc.sync.dma_start(gt[:ncpg], d_gcpg[mo:mo + ncpg, :])
                    b2t = smallp.tile([128, 1], F32, name="b2t")
                    nc.sync.dma_start(b2t[:ncpg], d_vb2[mo:mo + ncpg, :])
                    t2 = evp.tile([128, B], F32, name="t2")
                    nc.vector.scalar_tensor_tensor(
                        t2[:ncpg], y1[:ncpg], gt[:ncpg], xc[:ncpg], op0=mul, op1=mul)
                    y2 = evp.tile([128, B], F32, name="y2")
                    nc.scalar.activation(y2[:ncpg], t2[:ncpg], Relu, bias=b2t[:ncpg])
                    nc.sync.dma_start(y2sh[mo:mo + ncpg, :], y2[:ncpg])
                if mo + mw > SH_C:
                    fc0 = max(SH_C - mo, 0)
                    # partition offset 104 is not 32-aligned -> engines can't
                    # read it; DMA has free partition granularity
                    nc.sync.dma_start(snp_fc[:, :], y1[fc0:mw])

            y2full = dp.tile([CPG, B], F32, name="y2full", addr_space="Shared")
            nc.gpsimd.collective_compute(
                "AllGather", mybir.AluOpType.bypass, replica_groups=RG,
                ins=[y2sh[:]], outs=[y2full[:]])

            # ---------------- Stage B: fY2 -> Y3 (+cpg_fc), fp32 ----------------
            psB = [pp.tile([128, B], F32, tag=f"pb{j}", name=f"psB{j}")
                   for j in range(3)]
            for ki, (ko, kw) in enumerate(KB):
                wt = wpool.tile([128, AW], F32, name="wt")
                mt = mpool.tile([128, AW], F32, name="mt")
                nc.sync.dma_start(wt[:kw, :BW], d_bw[ko:ko + kw, :])
                nc.sync.dma_start(mt[:kw, :BW], d_bm[ko:ko + kw, :])
                nc.vector.tensor_mul(wt[:kw, :BW], wt[:kw, :BW], mt[:kw, :BW])
                yt = rhsp.tile([128, B], F32, name="xt")
                nc.sync.dma_start(yt[:kw], y2full[ko:ko + kw, :])
                st = ki == 0
                for j, (mo, mw) in enumerate(MB):
                    nc.tensor.matmul(psB[j][:mw], wt[:kw, mo:mo + mw], yt[:kw],
                                     start=st, stop=(j < 2 and ki == len(KB) - 1))
            # snp_fc contribution to cpg_fc (K=20 tail of the contraction)
            bw2t = smallp.tile([FC, FC], F32, name="bw2t")
            nc.sync.dma_start(bw2t[:], d_bw2[:])
            nc.tensor.matmul(psB[2][:FC], bw2t[:], snp_fc[:], start=False, stop=True)

            y3sh = dp.tile([SH_G, B], BF, name="y3sh")
            for j, (mo, mw) in enumerate(MB[:2]):
                beps = smallp.tile([128, 1], F32, name="beps")
                nc.sync.dma_start(beps[:mw], d_bb[mo:mo + mw, :])
                t = evp.tile([128, B], F32, name="y1")
                nc.scalar.activation(t[:mw], psB[j][:mw], Ident, bias=beps[:mw])
                r = evp.tile([128, B], F32, name="xc")
                nc.vector.reciprocal(r[:mw], t[:mw])
                xg = evp.tile([128, B], F32, name="t2")
                nc.sync.dma_start(xg[:mw], d_xgeneT[mo:mo + mw, :])
                ggt = smallp.tile([128, 1], F32, name="gt")
                nc.sync.dma_start(ggt[:mw], d_gg[mo:mo + mw, :])
                b3t = smallp.tile([128, 1], F32, name="b2t")
                nc.sync.dma_start(b3t[:mw], d_vb3[mo:mo + mw, :])
                pre = evp.tile([128, B], F32, name="y2")
                nc.vector.scalar_tensor_tensor(
                    pre[:mw], r[:mw], ggt[:mw], xg[:mw], op0=mul, op1=mul)
                y3 = evp.tile([128, B], BF, name="y3")
                nc.scalar.activation(y3[:mw], pre[:mw], Relu, bias=b3t[:mw])
                nc.sync.dma_start(y3sh[mo:mo + mw, :], y3[:mw])
            bcf = smallp.tile([FC, 1], F32, name="bcf")
            nc.sync.dma_start(bcf[:], d_bb[SH_G:BW, :])
            cpg_fc = pers.tile([FC, B], BF, name="cpg_fc")
            nc.scalar.activation(cpg_fc[:], psB[2][:FC], Relu, bias=bcf[:])

            y3full = dp.tile([GENE, B], BF, name="y3full", addr_space="Shared")
            nc.gpsimd.collective_compute(
                "AllGather", mybir.AluOpType.bypass, replica_groups=RG,
                ins=[y3sh[:]], outs=[y3full[:]])

            # ---------------- Stage C: Y4 (+gene_fc), bf16 ----------------
            psC = [pp.tile([128, B], F32, tag=f"pb{j}", name=f"psC{j}")
                   for j in range(2)]
            for ki, (ko, kw) in enumerate(KC):
                wt = wpool.tile([128, CW], BF, name="cwt")
                mt = mpool.tile([128, CW], BF, name="cmt")
                nc.sync.dma_start(wt[:kw], d_cw[ko:ko + kw, :])
                nc.sync.dma_start(mt[:kw], d_cm[ko:ko + kw, :])
                nc.vector.tensor_mul(wt[:kw], wt[:kw], mt[:kw])
                yt = rhsp.tile([128, B], BF, name="ytb")
                nc.sync.dma_start(yt[:kw], y3full[ko:ko + kw, :])
                st = ki == 0
                for j, (mo, mw) in enumerate(MC):
                    nc.tensor.matmul(psC[j][:mw], wt[:kw, mo:mo + mw], yt[:kw],
                                     start=st, stop=(j == 0 and ki == len(KC) - 1))
            cw2t = smallp.tile([FC, FC], BF, name="cw2t")
            nc.sync.dma_start(cw2t[:], d_cw2[:])
            nc.tensor.matmul(psC[1][:FC], cw2t[:], cpg_fc[:], start=False, stop=True)

            y4sh = dp.tile([SH_O, B], BF, name="y4sh")
            cb0 = smallp.tile([128, 1], F32, name="cb0")
            nc.sync.dma_start(cb0[:SH_O], d_cb[:SH_O, :])
            y4 = evp.tile([128, B], BF, name="y3")
            nc.scalar.activation(y4[:SH_O], psC[0][:SH_O], Relu, bias=cb0[:SH_O])
            nc.sync.dma_start(y4sh[:], y4[:SH_O])
            cbf = smallp.tile([FC, 1], F32, name="cbf")
            nc.sync.dma_start(cbf[:], d_cb[SH_O:CW, :])
            gene_fc = pers.tile([FC, B], BF, name="gene_fc")
            nc.scalar.activation(gene_fc[:], psC[1][:FC], Relu, bias=cbf[:])

            y4full = dp.tile([GO, B], BF, name="y4full", addr_space="Shared")
            nc.gpsimd.collective_compute(
                "AllGather", mybir.AluOpType.bypass, replica_groups=RG,
                ins=[y4sh[:]], outs=[y4full[:]])

            # ---------------- Tail: dense blocks, bf16 (replicated) ----------------
            KT = _ktiles(GO)  # 8 tiles: 7x128 + 104
            psT = pp.tile([128, B], F32, tag="pb7", name="psT")
            for ki, (ko, kw) in enumerate(KT):
                wt = wpool.tile([128, 128], BF, name="twt")
                nc.sync.dma_start(wt[:kw], d_w1[ko:ko + kw, :])
                yt = rhsp.tile([128, B], BF, name="ytb")
                nc.sync.dma_start(yt[:kw], y4full[ko:ko + kw, :])
                nc.tensor.matmul(psT[:], wt[:kw], yt[:kw], start=(ki == 0), stop=False)
            w1f = smallp.tile([FC, 128], BF, name="w1f")
            nc.sync.dma_start(w1f[:], d_w1[GO:GO + FC, :])
            nc.tensor.matmul(psT[:], w1f[:], gene_fc[:], start=False, stop=True)

            def block(ps_in, wdram, bdram, kdim, rhs_tile, out_dtype=BF, mdim=128,
                      tag_p="pb6", func=Relu):
                bt = smallp.tile([128, 1], F32, name="btb")
                nc.sync.dma_start(bt[:mdim], bdram[:])
                h = evp.tile([128, B], out_dtype, name="htl")
                nc.scalar.activation(h[:mdim], ps_in[:mdim], func, bias=bt[:mdim])
                return h

            h1 = block(psT, None, d_tb1, None, None)
            ps2 = pp.tile([128, B], F32, tag="pb6", name="ps2")
            wt2 = smallp.tile([128, 128], BF, name="wt2")
            nc.sync.dma_start(wt2[:], d_w2[:])
            nc.tensor.matmul(ps2[:], wt2[:], h1[:], start=True, stop=True)
            h2 = block(ps2, None, d_tb2, None, None)

            ps3 = pp.tile([128, B], F32, tag="pb5", name="ps3")
            wt3 = smallp.tile([128, 128], BF, name="wt3")
            nc.sync.dma_start(wt3[:], d_w3[:])
            nc.tensor.matmul(ps3[:], wt3[:], h2[:], start=True, stop=True)
            h3 = block(ps3, None, d_tb3, None, None)

            ps4 = pp.tile([128, B], F32, tag="pb6", name="ps4")
            wt4 = smallp.tile([128, 128], BF, name="wt4")
            nc.sync.dma_start(wt4[:], d_w4[:])
            nc.tensor.matmul(ps4[:], wt4[:], h3[:], start=True, stop=True)
            h4 = block(ps4, None, d_tb4, None, None)

            psp = pp.tile([128, B], F32, tag="pb5", name="psp")
            wtp = smallp.tile([128, FC], BF, name="wtp")
            nc.sync.dma_start(wtp[:], d_wp[:])
            nc.tensor.matmul(psp[:FC], wtp[:], h4[:], start=True, stop=True)
            btp = smallp.tile([FC, 1], F32, name="btp")
            nc.sync.dma_start(btp[:], d_tbp[:])
            dmt = pers.tile([FC + DG, B], BF, name="dmt")
            nc.scalar.activation(dmt[:FC], psp[:FC], Relu, bias=btp[:])
            nc.sync.dma_start(dmt[FC:FC + DG, :], d_demogT[:])

            psd = pp.tile([128, B], F32, tag="pb6", name="psd")
            wtd = smallp.tile([FC + DG, 128], BF, name="wtd")
            nc.sync.dma_start(wtd[:], d_wd[:])
            nc.tensor.matmul(psd[:], wtd[:], dmt[:], start=True, stop=True)
            dh = block(psd, None, d_tbd, None, None)

            pso = pp.tile([128, B], F32, tag="pb5", name="pso")
            wto = smallp.tile([128, 1], BF, name="wto")
            nc.sync.dma_start(wto[:], d_wo[:])
            nc.tensor.matmul(pso[:1], wto[:], dh[:], start=True, stop=True)
            bto = smallp.tile([1, 1], F32, name="bto")
            nc.sync.dma_start(bto[:], d_tbo[:])
            oo = evp.tile([1, B], F32, name="oo")
            nc.scalar.activation(oo[:], pso[:1], Ident, bias=bto[:])
            nc.sync.dma_start(d_out[:], oo[:])

    nc.compile()
    return nc


_NC_CACHE = {}


def _get_nc():
    if "nc" not in _NC_CACHE:
        _NC_CACHE["nc"] = build_nc()
    return _NC_CACHE["nc"]


def _fold_bn(bp):
    bn = bp["bn"]
    s = (bn["gamma"] / np.sqrt(bn["var"] + 1e-5)).astype(np.float32)
    t = (bn["beta"] - bn["mean"] * s).astype(np.float32)
    W = np.asarray(bp["lin"]["W"], np.float32)
    b = np.asarray(bp["lin"]["b"], np.float32)
    Wf = s[:, None] * W
    bf_ = t @ W + b
    return Wf, bf_


def _prep_in_maps(x_snp, x_cpg, x_gene, demog, mask_sc, mask_cg, mask_gg, params):
    def _tonp(d):
        return {k: (_tonp(v) if isinstance(v, dict) else np.asarray(v))
                for k, v in d.items()}
    p = _tonp(params)
    C = np.ascontiguousarray

    xT = C(np.asarray(x_snp).T.astype(np.float32))
    xcpgT = C(np.asarray(x_cpg).T.astype(np.float32))
    xgeneT = C(np.asarray(x_gene).T.astype(np.float32))
    demogT = C(np.asarray(demog).T.astype(BF16))

    W_sc = np.asarray(p["snp_to_cpg"]["W"], np.float32)
    b_sc = np.asarray(p["snp_to_cpg"]["b"], np.float32)
    W_fc = np.asarray(p["snp_fc20"]["W"], np.float32)
    b_fc = np.asarray(p["snp_fc20"]["b"], np.float32)
    W_cg = np.asarray(p["cpg_to_gene"]["W"], np.float32)
    b_cg = np.asarray(p["cpg_to_gene"]["b"], np.float32)
    W_cf = np.asarray(p["cpg_fc20"]["W"], np.float32)
    b_cf = np.asarray(p["cpg_fc20"]["b"], np.float32)
    W_gg = np.asarray(p["gene_to_go"]["W"], np.float32)
    b_gg = np.asarray(p["gene_to_go"]["b"], np.float32)
    W_gf = np.asarray(p["gene_fc20"]["W"], np.float32)
    b_gf = np.asarray(p["gene_fc20"]["b"], np.float32)
    m_sc = np.asarray(mask_sc, np.float32)
    m_cg = np.asarray(mask_cg, np.float32)
    m_gg = np.asarray(mask_gg, np.float32)
    g_cpg = np.asarray(p["g_cpg"], np.float32)
    B2v = np.asarray(p["B2"], np.float32)
    g_gene = np.asarray(p["g_gene"], np.float32)
    B3v = np.asarray(p["B3"], np.float32)

    w1, tb1 = _fold_bn(p["block1"])
    w2, tb2 = _fold_bn(p["block2"])
    w3, tb3 = _fold_bn(p["block3"])
    w4, tb4 = _fold_bn(p["block4"])
    wd, tbd = _fold_bn(p["demog_block"])
    wp_ = np.asarray(p["post_dense20"]["W"], np.float32)
    tbp = np.asarray(p["post_dense20"]["b"], np.float32)
    wo = np.asarray(p["output"]["W"], np.float32)
    tbo = np.asarray(p["output"]["b"], np.float32)

    ones_snp = np.ones((SNP, FC), np.float32)
    ones_cpg = np.ones((CPG, FC), np.float32)
    ones_gene = np.ones((GENE, FC), np.float32)

    in_maps = []
    for i in range(NCORES):
        sc = slice(i * SH_C, (i + 1) * SH_C)
        sg = slice(i * SH_G, (i + 1) * SH_G)
        so = slice(i * SH_O, (i + 1) * SH_O)
        m = {
            "xT": xT,
            "aw": C(np.concatenate([W_sc[:, sc], W_fc], axis=1)),
            "am": C(np.concatenate([m_sc[:, sc], ones_snp], axis=1)),
            "ab": C(np.concatenate([b_sc[sc], b_fc]).reshape(-1, 1)),
            "xcpgT": C(xcpgT[sc]),
            "gcpg": C(g_cpg[sc].reshape(-1, 1)),
            "vb2": C(B2v[sc].reshape(-1, 1)),
            "bw": C(np.concatenate([W_cg[:, sg], W_cf[:CPG]], axis=1)),
            "bm": C(np.concatenate([m_cg[:, sg], ones_cpg], axis=1)),
            "bb": C(np.concatenate([b_cg[sg] + np.float32(EPS), b_cf]).reshape(-1, 1)),
            "bw2": C(W_cf[CPG:CPG + FC]),
            "xgeneT": C(xgeneT[sg]),
            "gg": C(g_gene[sg].reshape(-1, 1)),
            "vb3": C(B3v[sg].reshape(-1, 1)),
            "cw": C(np.concatenate([W_gg[:, so], W_gf[:GENE]], axis=1).astype(BF16)),
            "cm": C(np.concatenate([m_gg[:, so], ones_gene], axis=1).astype(BF16)),
            "cb": C(np.concatenate([b_gg[so], b_gf]).reshape(-1, 1)),
            "cw2": C(W_gf[GENE:GENE + FC].astype(BF16)),
            "w1": C(w1.astype(BF16)),
            "tb1": C(tb1.reshape(-1, 1)),
            "w2": C(w2.astype(BF16)),
            "tb2": C(tb2.reshape(-1, 1)),
            "w3": C(w3.astype(BF16)),
            "tb3": C(tb3.reshape(-1, 1)),
            "w4": C(w4.astype(BF16)),
            "tb4": C(tb4.reshape(-1, 1)),
            "wp": C(wp_.astype(BF16)),
            "tbp": C(tbp.reshape(-1, 1)),
            "demogT": demogT,
            "wd": C(wd.astype(BF16)),
            "tbd": C(tbd.reshape(-1, 1)),
            "wo": C(wo.astype(BF16)),
            "tbo": C(tbo.reshape(1, 1)),
        }
        in_maps.append(m)
    return in_maps


def kernel(x_snp, x_cpg, x_gene, demog, mask_sc, mask_cg, mask_gg, params,
           _trace=False):
    nc = _get_nc()
    in_maps = _prep_in_maps(x_snp, x_cpg, x_gene, demog,
                            mask_sc, mask_cg, mask_gg, params)
    res = run_bass_kernel_spmd(nc, in_maps, list(range(NCORES)), trace=_trace)
    out = res.results[0]["out"].reshape(B).astype(np.float32)
    if _trace:
        kernel._last_exec_time_ns = res.exec_time_ns
        kernel._last_profile = res
    return out
